# revision 28
# baseline (speedup 1.0000x reference)
"""EnhancedTransformerBlock on 8 TRN2 NeuronCores.

Strategy: pure data-parallel over batch (B=8 -> 1 element/core, no
collectives). Per core the block runs in "T-layout" ([feature, token],
features on partitions) so every matmul contracts over the partition dim.

- x is transposed on the HOST and shipped both as fp8 (matmul path) and
  f32 (residual path; DMA'd during attention).
- All weights are host-packed fp8e4 (scaled by 128) in chunk-contiguous
  DMA layouts (1-4KB per-partition lines instead of 128B descriptors).
- All weight-stationary matmuls except the attention score matmuls run as
  fp8 DoubleRow (2 k-chunks per matmul): QKV, AV, Wo, gate1/2, ffn w1/w2.
  Unscaling is folded into eviction scale= operands.
- Attention is software-pipelined: pair hp's scores/exp interleave with
  pair hp-1's AV at matmul granularity; the e-pool holds 2 pairs in
  flight so ScalarE (exp, the region bottleneck) never stalls on AV.
  Rowsum reciprocal rows are broadcast across partitions by GpSimd
  (partition_broadcast) instead of K=1 PE matmuls. ctx is carried at 16x
  in fp8 (subnormal avoidance), unscaled in the Wo eviction.
- LayerNorm1 in T-layout: column sums via ones-column matmuls, per-token
  scale/shift broadcast across partitions with K=1 matmuls.
- The residual trunk x2 (accf) is bf16; LayerNorm2 runs in N-layout after
  bf16 PE transposes (evictions alternate VectorE/ScalarE), interleaved
  per q-half with the second ffn_w2 pass so the tail overlaps matmuls.
  Typical HW exec: ~505-525us (baseline 704us); rel err ~1.03e-2.
"""
import sys

sys.path.insert(0, '/opt/trn_rl_repo')

import numpy as np
import ml_dtypes

import concourse.bass as bass
import concourse.bacc as bacc
import concourse.tile as tile
from concourse import mybir
from concourse.bass_utils import run_bass_kernel_spmd
from concourse.masks import make_identity

F32 = mybir.dt.float32
BF16 = mybir.dt.bfloat16
F8 = mybir.dt.float8e4
AF = mybir.ActivationFunctionType
OP = mybir.AluOpType
DR = mybir.MatmulPerfMode.DoubleRow

P = 128
B, S, H = 8, 1024, 1024
NH, HD = 16, 64
HF, HG = 4 * H, H // 2
HC = H // P          # 8 feature chunks
FC = HF // P         # 32 ffn chunks
GC = HG // P         # 4 gate chunks
QT = S // 512        # 2 q tiles of 512
EPS = 1e-5
WSC = 128.0          # host-side weight scale (fp8 subnormal avoidance)
WS = 1.0 / WSC
CTXS = 16.0          # ctx carried at 16x in fp8

_BUILD_CACHE = {}


def _bcast_ap(param, n_part, n_free):
    """AP reading a [n_free] DRAM tensor broadcast across n_part partitions."""
    ap = param[None, :]
    return bass.AP(tensor=ap.tensor, offset=ap.offset, ap=[[0, n_part], [1, n_free]])


def _build(flags):
    f = dict(flags)
    nc = bacc.Bacc(None, target_bir_lowering=False)

    dp = nc.declare_dram_parameter
    xt8 = dp("xt8", [P, HC * S], F8, isOutput=False)
    xtf = dp("xtf", [P, HC * S], F32, isOutput=False)
    vol = dp("vol", [S], F32, isOutput=False)
    wq = dp("wq", [HC, P, H], F8, isOutput=False)
    wk = dp("wk", [HC, P, H], F8, isOutput=False)
    wv = dp("wv", [2, P, HC * 512], F8, isOutput=False)
    wo = dp("wo", [HC, P, H], F8, isOutput=False)
    w1 = dp("w1", [FC, P, H], F8, isOutput=False)
    w2 = dp("w2", [2, P, FC, 512], F8, isOutput=False)
    g1 = dp("g1", [GC, P, H], F8, isOutput=False)
    g2 = dp("g2", [HC, P, HG], F8, isOutput=False)
    bq = dp("bq", [H], F32, isOutput=False)
    bk = dp("bk", [H], F32, isOutput=False)
    bv = dp("bv", [H], F32, isOutput=False)
    bo = dp("bo", [H], F32, isOutput=False)
    b1 = dp("b1", [HF], F32, isOutput=False)
    b2 = dp("b2", [H], F32, isOutput=False)
    gb1 = dp("gb1", [HG], F32, isOutput=False)
    gb2 = dp("gb2", [H], F32, isOutput=False)
    ln1w = dp("ln1w", [H], F32, isOutput=False)
    ln1b = dp("ln1b", [H], F32, isOutput=False)
    ln2w = dp("ln2w", [H], F32, isOutput=False)
    ln2b = dp("ln2b", [H], F32, isOutput=False)
    sc = {}
    for name in ("gamma1", "beta1", "vs1w", "vs1b", "gamma2", "beta2", "vs2w", "vs2b"):
        sc[name] = dp(name, [1], F32, isOutput=False)
    out = dp("out", [S, H], F32, isOutput=True)

    def chunked(param):  # [n] f32 -> [P, n//P] per-partition layout
        return param.rearrange("(c p) -> p c", p=P)

    with tile.TileContext(nc) as tc:
        from contextlib import ExitStack
        with ExitStack() as ctx:
            const = ctx.enter_context(tc.tile_pool(name="const", bufs=1))

            identb = const.tile([P, P], BF16)
            make_identity(nc, identb)
            ones_col = const.tile([P, 1], BF16)
            nc.vector.memset(ones_col, 1.0)
            ones_f8 = const.tile([P, 1], F8)
            nc.vector.memset(ones_f8, 1.0)
            ones_row = const.tile([1, P], BF16)
            nc.vector.memset(ones_row, 1.0)
            eps128 = const.tile([P, 1], F32)
            nc.vector.memset(eps128, EPS)

            # persistent slabs, tag-shared across phases
            trunk = ctx.enter_context(tc.tile_pool(name="trunk", bufs=1))
            xTf = trunk.tile([P, HC, S], F32, tag="f4a", name="xTf")  # x^T -> x1 -> y1
            QTs = trunk.tile([P, HC, S], BF16, tag="bf2e", name="QTs")
            KTs = trunk.tile([P, HC, S], BF16, tag="bf2b", name="KTs")
            Vp = trunk.tile([P, HC, NH, HD + 1], F8, tag="bf2c", name="Vp")
            xT8 = trunk.tile([P, HC, S], F8, tag="bf2d", name="xT8")

            # host-pretransposed x, fp8 matmul copy (per-chunk DMAs)
            for c in range(HC):
                nc.sync.dma_start(out=xT8[:, c, :], in_=xt8[:, c * S:(c + 1) * S])

            def load_chunked(param, n):
                t = const.tile([P, n], F32, name=f"c_{param.name}")
                nc.sync.dma_start(out=t, in_=chunked(param))
                return t

            bq_sb = load_chunked(bq, HC)
            bk_sb = load_chunked(bk, HC)
            bo_sb = load_chunked(bo, HC) if f["bo"] else None
            b1_sb = load_chunked(b1, FC)
            b2_sb = load_chunked(b2, HC) if f["b2"] else None
            gb1_sb = load_chunked(gb1, GC)
            gb2_sb = load_chunked(gb2, HC)
            if f["bv"]:
                bv_bc = const.tile([P, H], F32)
                nc.gpsimd.dma_start(out=bv_bc, in_=_bcast_ap(bv, P, H))
            if f["ln1w"]:
                ln1w_sb = load_chunked(ln1w, HC)
            if f["ln1b"]:
                ln1b_sb = load_chunked(ln1b, HC)
            if f["ln2w"]:
                ln2w_bc = const.tile([P, H], F32)
                nc.gpsimd.dma_start(out=ln2w_bc, in_=_bcast_ap(ln2w, P, H))
            if f["ln2b"]:
                ln2b_bc = const.tile([P, H], F32)
                nc.gpsimd.dma_start(out=ln2b_bc, in_=_bcast_ap(ln2b, P, H))

            sct = {}
            for name in ("gamma1", "vs1w", "vs1b"):
                t = const.tile([1, 1], F32, name=f"sc_{name}")
                nc.sync.dma_start(out=t, in_=sc[name][None, :])
                sct[name] = t
            for name in ("gamma2", "beta2", "vs2w", "vs2b", "beta1"):
                t = const.tile([P, 1], F32, name=f"sc_{name}")
                nc.gpsimd.dma_start(out=t, in_=_bcast_ap(sc[name], P, 1))
                sct[name] = t

            # volatility-derived per-token scales
            vol_row = const.tile([1, S], F32)
            nc.sync.dma_start(out=vol_row, in_=vol[None, :])
            s1row = const.tile([1, S], F32)
            nc.scalar.activation(s1row, vol_row, AF.Sigmoid,
                                 bias=sct["vs1b"][0:1, :], scale=sct["vs1w"][0:1, :])
            nc.vector.tensor_scalar(s1row, s1row, 1.0, sct["gamma1"],
                                    op0=OP.add, op1=OP.mult)
            vol_np = const.tile([P, HC], F32)
            nc.sync.dma_start(out=vol_np, in_=chunked(vol))
            s2_np = const.tile([P, HC], F32)
            nc.scalar.activation(s2_np, vol_np, AF.Sigmoid,
                                 bias=sct["vs2b"], scale=sct["vs2w"])
            nc.vector.tensor_scalar(s2_np, s2_np, 1.0, sct["gamma2"],
                                    op0=OP.add, op1=OP.mult)

            # ---------------- P2: Q/K projections (fp8 DoubleRow) ----------
            nc.vector.memset(Vp[:, :, :, HD:HD + 1], 1.0)
            p2w = ctx.enter_context(tc.tile_pool(name="p2w", bufs=3))
            p2wv = ctx.enter_context(tc.tile_pool(name="p2wv", bufs=2))
            ps_ctx = ExitStack()
            p23ps = ps_ctx.enter_context(
                tc.tile_pool(name="p23ps", bufs=2, space="PSUM"))

            def emit_qk_pair(mc):
                for w_par, dst, bias_sb in ((wq, QTs, bq_sb), (wk, KTs, bk_sb)):
                    wt = p2w.tile([P, HC, P], F8, tag="wproj", name="wt_qk")
                    nc.sync.dma_start(out=wt, in_=w_par[mc])
                    for qt in range(QT):
                        ps = p23ps.tile([P, 512], F32, tag="ps_qv", name="ps_qk")
                        for g in range(HC // 2):
                            nc.tensor.matmul(ps, wt[:, 2 * g:2 * g + 2, :],
                                             xT8[:, 2 * g:2 * g + 2,
                                                 qt * 512:(qt + 1) * 512],
                                             perf_mode=DR,
                                             start=(g == 0), stop=(g == HC // 2 - 1))
                        nc.vector.tensor_scalar(
                            dst[:, mc, qt * 512:(qt + 1) * 512], ps, WS,
                            bias_sb[:, mc:mc + 1], op0=OP.mult, op1=OP.add)

            def emit_v(dt):
                wt = p2wv.tile([P, HC, 512], F8, tag="wv")
                nc.sync.dma_start(out=wt, in_=wv[dt])
                for kc in range(HC):
                    ps = p23ps.tile([P, 512], F32, tag="ps_qv")
                    for g in range(HC // 2):
                        nc.tensor.matmul(ps,
                                         xT8[:, 2 * g:2 * g + 2, kc * P:(kc + 1) * P],
                                         wt[:, 2 * g:2 * g + 2, :],
                                         perf_mode=DR,
                                         start=(g == 0), stop=(g == HC // 2 - 1))
                    dst = Vp[:, kc, dt * 8:(dt + 1) * 8, 0:HD]
                    src = ps.rearrange("p (h d) -> p h d", d=HD)
                    if f["bv"]:
                        nc.vector.scalar_tensor_tensor(
                            dst, src, WS,
                            bv_bc[:, dt * 512:(dt + 1) * 512].rearrange(
                                "p (h d) -> p h d", d=HD),
                            op0=OP.mult, op1=OP.add)
                    else:
                        nc.vector.tensor_scalar(dst, src, WS, None, op0=OP.mult)

            for mc in range(HC):
                emit_qk_pair(mc)
            emit_v(0)
            # f32 x^T for the residual path (DMA runs during attention, ahead
            # of the Wo weight loads in queue order)
            for c in range(HC):
                nc.sync.dma_start(out=xTf[:, c, :], in_=xtf[:, c * S:(c + 1) * S])

            # ---------------- P3: attention (software-pipelined) -----------
            # ctxT gets its own slab: its writes start while xT8 is still
            # being read by late Q/K/V matmuls (slab sharing would WAR-block)
            ctxT = trunk.tile([P, HC, S], F8, tag="ctx", name="ctxT")
            with tc.tile_pool(name="p3e", bufs=4) as p3e, \
                 tc.tile_pool(name="p3r", bufs=2) as p3r:

                def emit_av_item(h, e, qt):
                    p0 = 64 * (h % 2)
                    pav = p23ps.tile([65, 512], F32, tag="ps_av", bufs=2)
                    for g in range(HC // 2):
                        nc.tensor.matmul(pav,
                                         Vp[:, 2 * g:2 * g + 2, h, :],
                                         e[:, 2 * g:2 * g + 2,
                                           qt * 512:(qt + 1) * 512],
                                         perf_mode=DR,
                                         start=(g == 0), stop=(g == HC // 2 - 1))
                    # reciprocal of the rowsum row, broadcast across
                    # partitions by GpSimd, then normalize (ctx at 16x)
                    rs = p3r.tile([1, 512], F32, tag="rsum")
                    nc.vector.tensor_scalar(rs, pav[64:65, :], 1.0 / CTXS,
                                            None, op0=OP.mult)
                    rrow = p3r.tile([1, 512], F32, tag="rrow")
                    nc.vector.reciprocal_approx_fast(out=rrow, in_=rs)
                    rec = p3r.tile([64, 512], F32, tag="rec")
                    nc.gpsimd.partition_broadcast(rec, rrow, channels=64)
                    nc.vector.tensor_tensor(
                        ctxT[p0:p0 + 64, h // 2, qt * 512:(qt + 1) * 512],
                        rec, pav[0:64, :], OP.mult)

                prev = None
                for hp in range(NH // 2):
                    pair = [(2 * hp, p3e.tile([P, HC, S], F8, tag="E", name="e0")),
                            (2 * hp + 1, p3e.tile([P, HC, S], F8, tag="E", name="e1"))]
                    avq = ([(h, e, qt) for h, e in prev for qt in range(QT)]
                           if prev else [])
                    for kc in range(HC):
                        pss = [p23ps.tile([P, S], F32, tag="ps_s", bufs=2,
                                          name=f"ps_s{i}")
                               for i in range(len(pair))]
                        for qt in range(QT):
                            for i, (h, e) in enumerate(pair):
                                p0 = 64 * (h % 2)
                                nc.tensor.matmul(
                                    pss[i][:, qt * 512:(qt + 1) * 512],
                                    KTs[p0:p0 + 64, hp, kc * P:(kc + 1) * P],
                                    QTs[p0:p0 + 64, hp, qt * 512:(qt + 1) * 512],
                                    start=True, stop=True)
                        for i, (h, e) in enumerate(pair):
                            nc.scalar.activation(e[:, kc, :], pss[i], AF.Exp,
                                                 scale=0.125)
                        if kc % 2 == 1 and avq:
                            emit_av_item(*avq.pop(0))
                    if hp == 0:
                        emit_v(1)  # PE filler while exp(pair 0) drains
                    for item in avq:
                        emit_av_item(*item)
                    prev = pair
                for h, e in prev:
                    for qt in range(QT):
                        emit_av_item(h, e, qt)
            ps_ctx.close()  # release P2/P3 PSUM banks before P4

            # ---------------- P4+P5: Wo + residual + LN1 + gate ----------------
            x1f8 = trunk.tile([P, HC, S], F8, tag="bf2a", name="x1f8")
            gT = trunk.tile([P, HC, S], BF16, tag="bf2e", name="gT")
            rT = trunk.tile([P, GC, S], F8, tag="bf2c", name="rT")
            g1_bufs = 1 if f["ln1b"] else 2
            with tc.tile_pool(name="pw", bufs=3) as pw:
              with tc.tile_pool(name="pt4", bufs=1) as pt4, \
                   tc.tile_pool(name="pAps", bufs=1, space="PSUM") as pAps:
                for qt in range(QT):
                    sl = slice(qt * 512, (qt + 1) * 512)
                    for mc in range(HC):
                        wt = pw.tile([P, HC, P], F8, tag="wproj", bufs=3)
                        nc.sync.dma_start(out=wt, in_=wo[mc])
                        ps = pAps.tile([P, 512], F32, tag="ps_o", bufs=2)
                        for g in range(HC // 2):
                            nc.tensor.matmul(ps, wt[:, 2 * g:2 * g + 2, :],
                                             ctxT[:, 2 * g:2 * g + 2,
                                                  qt * 512:(qt + 1) * 512],
                                             perf_mode=DR,
                                             start=(g == 0), stop=(g == HC // 2 - 1))
                        xs = xTf[:, mc, sl]
                        nc.vector.scalar_tensor_tensor(xs, ps, WS / CTXS, xs,
                                                       op0=OP.mult, op1=OP.add)
                        if f["bo"]:
                            nc.vector.tensor_scalar(xs, xs, bo_sb[:, mc:mc + 1], None,
                                                    op0=OP.add)
                        nc.scalar.activation(x1f8[:, mc, sl], xs, AF.Identity)
                    # LN1 for this q-tile; xTf: x1 -> y1 in place
                    pstat = pAps.tile([33, 512], F32, tag="ps_stat")
                    for mc in range(HC):
                        nc.tensor.matmul(pstat[0:1, :], ones_f8, x1f8[:, mc, sl],
                                         start=(mc == 0), stop=(mc == HC - 1))
                    sq = pt4.tile([P, HC, 512], BF16, tag="sq")
                    nc.scalar.activation(sq, xTf[:, :, sl], AF.Square)
                    for mc in range(HC):
                        nc.tensor.matmul(pstat[32:33, :], ones_col, sq[:, mc, :],
                                         start=(mc == 0), stop=(mc == HC - 1))
                    mu = pt4.tile([1, 512], F32, tag="mu")
                    nc.vector.tensor_scalar(mu, pstat[0:1, :], 1.0 / H, None, op0=OP.mult)
                    mu2 = pt4.tile([1, 512], F32, tag="mu2")
                    nc.vector.tensor_tensor(mu2, mu, mu, OP.mult)
                    var = pt4.tile([1, 512], F32, tag="var")
                    # var = sumsq/H - mu^2 in one op
                    nc.vector.scalar_tensor_tensor(var, pstat[32:33, :], 1.0 / H, mu2,
                                                   op0=OP.mult, op1=OP.subtract)
                    nc.scalar.activation(var, var, AF.Sqrt, bias=eps128[0:1, :])
                    rstd = pt4.tile([1, 512], F32, tag="rstd")
                    nc.vector.reciprocal_approx_fast(out=rstd, in_=var)
                    arow = pt4.tile([1, 512], F32, tag="arow")
                    nc.vector.tensor_tensor(arow, rstd, s1row[0:1, sl], OP.mult)
                    arow_bf = pt4.tile([1, 512], BF16, tag="arow_bf")
                    nc.vector.tensor_copy(arow_bf, arow)
                    crow_bf = pt4.tile([1, 512], BF16, tag="crow_bf")
                    nc.vector.tensor_tensor(crow_bf, mu, arow, OP.mult)
                    psa = pAps.tile([P, 512], F32, tag="ps_a")
                    nc.tensor.matmul(psa, ones_row, arow_bf, start=True, stop=True)
                    psc = pAps.tile([P, 512], F32, tag="ps_c")
                    nc.tensor.matmul(psc, ones_row, crow_bf, start=True, stop=True)
                    if f["ln1b"]:
                        s1_bf = pt4.tile([1, 512], BF16, tag="s1_bf")
                        nc.vector.tensor_copy(s1_bf, s1row[0:1, sl])
                        pss1 = pAps.tile([P, 512], F32, tag="ps_s1")
                        nc.tensor.matmul(pss1, ones_row, s1_bf, start=True, stop=True)
                    for mc in range(HC):
                        y = xTf[:, mc, sl]
                        nc.vector.tensor_tensor(y, y, psa, OP.mult)
                        nc.vector.tensor_tensor(y, y, psc, OP.subtract)
                        if f["ln1w"]:
                            nc.vector.tensor_scalar(y, y, ln1w_sb[:, mc:mc + 1], None,
                                                    op0=OP.mult)
                        if f["ln1b"]:
                            bs = pt4.tile([P, 512], F32, tag="bs")
                            nc.vector.tensor_scalar(bs, pss1, ln1b_sb[:, mc:mc + 1],
                                                    None, op0=OP.mult)
                            nc.vector.tensor_tensor(y, y, bs, OP.add)
                        if f["beta1"]:
                            nc.vector.tensor_scalar(y, y, sct["beta1"], None, op0=OP.add)
                        nc.scalar.activation(x1f8[:, mc, sl], y, AF.Identity)

                # gate first layer (runs while LN1 of the second q-tile drains)
                for qt in range(QT):
                    sl = slice(qt * 512, (qt + 1) * 512)
                    for mc in range(GC):
                        wt = pw.tile([P, HC, P], F8, tag="wproj", bufs=3)
                        nc.sync.dma_start(out=wt, in_=g1[mc])
                        ps = pAps.tile([P, 512], F32, tag="ps_g1", bufs=g1_bufs)
                        for g in range(HC // 2):
                            nc.tensor.matmul(ps, wt[:, 2 * g:2 * g + 2, :],
                                             x1f8[:, 2 * g:2 * g + 2, sl],
                                             perf_mode=DR,
                                             start=(g == 0), stop=(g == HC // 2 - 1))
                        nc.scalar.activation(rT[:, mc, sl], ps, AF.Relu,
                                             bias=gb1_sb[:, mc:mc + 1], scale=WS)

              y1f8 = x1f8  # fp8 y1; xTf holds f32 y1

              # ---------------- P6: gate2 + FFN + gated mix; P7 LN2 --------
              accf = trunk.tile([P, HC, S], BF16, tag="f4c", name="accf")
              with tc.tile_pool(name="pt7", bufs=2) as pt7, \
                   tc.tile_pool(name="pCps", bufs=1, space="PSUM") as pCps:
                    psk = [0]

                    def accps(shape, dtype=F32):
                        t = pCps.tile(shape, dtype, tag=f"ps_acc{psk[0] % 4}",
                                      name=f"psacc{psk[0] % 4}")
                        psk[0] += 1
                        return t

                    uT = pt7.tile([P, HC, S], BF16, tag="u", bufs=1, name="uT")
                    for qt in range(QT):
                        for mc in range(HC):
                            wt = pw.tile([P, GC, P], F8, tag="wg2", bufs=3)
                            nc.sync.dma_start(out=wt, in_=g2[mc])
                            ps = accps([P, 512])
                            for g in range(GC // 2):
                                nc.tensor.matmul(ps, wt[:, 2 * g:2 * g + 2, :],
                                                 rT[:, 2 * g:2 * g + 2,
                                                    qt * 512:(qt + 1) * 512],
                                                 perf_mode=DR,
                                                 start=(g == 0), stop=(g == GC // 2 - 1))
                            qsl = slice(qt * 512, (qt + 1) * 512)
                            nc.scalar.activation(gT[:, mc, qsl], ps,
                                                 AF.Sigmoid, bias=gb2_sb[:, mc:mc + 1],
                                                 scale=WS)
                            # u = y1*(2-g): precompute the residual half of the
                            # gated mix here, where the DVE is otherwise idle
                            tg = pt7.tile([P, 512], BF16, tag="tg", bufs=2)
                            nc.vector.tensor_scalar(tg, gT[:, mc, qsl], -1.0, 2.0,
                                                    op0=OP.mult, op1=OP.add)
                            nc.vector.tensor_tensor(uT[:, mc, qsl],
                                                    xTf[:, mc, qsl], tg, OP.mult)

                    def emit_w1(half):
                        hA = trunk.tile([P, 8, S], F8, tag="bf2b", name="hA")
                        hB = trunk.tile([P, 8, S], F8, tag="bf2d", name="hB")
                        for c in range(16):
                            cg = half * 16 + c
                            wt = pw.tile([P, HC, P], F8, tag="wproj", bufs=3)
                            nc.sync.dma_start(out=wt, in_=w1[cg])
                            psh = accps([P, S])
                            for qt in range(QT):
                                for g in range(HC // 2):
                                    nc.tensor.matmul(
                                        psh[:, qt * 512:(qt + 1) * 512],
                                        wt[:, 2 * g:2 * g + 2, :],
                                        y1f8[:, 2 * g:2 * g + 2,
                                             qt * 512:(qt + 1) * 512],
                                        perf_mode=DR,
                                        start=(g == 0), stop=(g == HC // 2 - 1))
                            dsth = hA[:, c, :] if c < 8 else hB[:, c - 8, :]
                            nc.scalar.activation(dsth, psh, AF.Gelu,
                                                 bias=b1_sb[:, cg:cg + 1], scale=WS)
                        return hA, hB

                    def hsl2(hA, hB, c, qsl):  # [P, 2, n] DoubleRow rhs slice
                        return (hA[:, c:c + 2, qsl] if c < 8
                                else hB[:, c - 8:c - 6, qsl])

                    def emit_w2_group(half, hA, hB, qt, oh):
                        qsl = slice(qt * 512, (qt + 1) * 512)
                        accs = [accps([P, 512]) for mc in range(4)]
                        for cp in range(8):
                            wt = pw.tile([P, 2, 512], F8, tag="w2", bufs=6)
                            nc.sync.dma_start(
                                out=wt,
                                in_=w2[oh, :, half * 16 + 2 * cp:half * 16 + 2 * cp + 2,
                                       :])
                            for mc in range(4):
                                nc.tensor.matmul(
                                    accs[mc],
                                    wt[:, :, mc * P:(mc + 1) * P],
                                    hsl2(hA, hB, 2 * cp, qsl),
                                    perf_mode=DR,
                                    start=(cp == 0), stop=(cp == 7))
                        for mc in range(4):
                            mcg = oh * 4 + mc
                            a = accf[:, mcg, qsl]
                            if half == 0:
                                # store ffn_half0
                                nc.vector.tensor_scalar(a, accs[mc], WS, None,
                                                        op0=OP.mult)
                            else:
                                # x2 = g*(ffn0+ffn1) + y1*(2-g); the second term
                                # (uT) was precomputed in the gate2 phase
                                af = pt7.tile([P, 512], BF16, tag="af", bufs=2)
                                nc.vector.scalar_tensor_tensor(
                                    af, accs[mc], WS, a, op0=OP.mult, op1=OP.add)
                                if f["b2"]:
                                    nc.vector.tensor_scalar(
                                        af, af, b2_sb[:, mcg:mcg + 1], None, op0=OP.add)
                                g = gT[:, mcg, qsl]
                                nc.vector.tensor_tensor(af, af, g, OP.mult)
                                nc.vector.tensor_tensor(a, af, uT[:, mcg, qsl],
                                                        OP.add)

                    def emit_ln2(qc):
                        xt = pt7.tile([P, H], BF16, tag="x2")
                        for hc in range(HC):
                            pst = accps([P, P], BF16)
                            nc.tensor.transpose(pst,
                                                accf[:, hc, qc * P:(qc + 1) * P],
                                                identb)
                            if hc % 2:
                                nc.scalar.activation(xt[:, hc * P:(hc + 1) * P],
                                                     pst, AF.Identity)
                            else:
                                nc.vector.tensor_copy(xt[:, hc * P:(hc + 1) * P],
                                                      pst)
                        stats = pt7.tile([P, 2, nc.vector.BN_STATS_DIM], F32,
                                         tag="stats")
                        for sg in range(2):
                            nc.vector.bn_stats(stats[:, sg, :],
                                               xt[:, sg * 512:(sg + 1) * 512])
                        mv = pt7.tile([P, nc.vector.BN_AGGR_DIM], F32, tag="mv")
                        nc.vector.bn_aggr(mv, stats)
                        sd = pt7.tile([P, 1], F32, tag="sd")
                        nc.scalar.activation(sd, mv[:, 1:2], AF.Sqrt, bias=eps128)
                        rstd2 = pt7.tile([P, 1], F32, tag="rstd2")
                        nc.vector.reciprocal(rstd2, sd)
                        a2 = pt7.tile([P, 1], F32, tag="a2")
                        nc.vector.tensor_tensor(a2, rstd2, s2_np[:, qc:qc + 1], OP.mult)
                        nmua = pt7.tile([P, 1], F32, tag="nmua")
                        nc.vector.scalar_tensor_tensor(nmua, mv[:, 0:1], -1.0, a2,
                                                       op0=OP.mult, op1=OP.mult)
                        ot = pt7.tile([P, H], F32, tag="ot")
                        nc.scalar.activation(ot, xt, AF.Identity, bias=nmua, scale=a2)
                        if f["ln2w"]:
                            nc.vector.tensor_tensor(ot, ot, ln2w_bc, OP.mult)
                        if f["ln2b"]:
                            bs2 = pt7.tile([P, H], F32, tag="bs2")
                            nc.vector.tensor_scalar(bs2, ln2b_bc, s2_np[:, qc:qc + 1],
                                                    None, op0=OP.mult)
                            nc.vector.tensor_tensor(ot, ot, bs2, OP.add)
                        if f["beta2"]:
                            nc.vector.tensor_scalar(ot, ot, sct["beta2"], None,
                                                    op0=OP.add)
                        nc.sync.dma_start(out=out[qc * P:(qc + 1) * P, :], in_=ot)

                    # half 0: w1 then w2 (qt-minor, unchanged)
                    hA, hB = emit_w1(0)
                    for oh in range(2):
                        for qt in range(QT):
                            emit_w2_group(0, hA, hB, qt, oh)
                    # half 1: w1, then w2 qt-major with LN2 interleaved so the
                    # first q-half's output drains while the second computes
                    hA, hB = emit_w1(1)
                    for qt in range(QT):
                        for oh in range(2):
                            emit_w2_group(1, hA, hB, qt, oh)
                        if qt == 0:
                            for qc in range(0, 4):
                                emit_ln2(qc)
                    for qc in range(4, 8):
                        emit_ln2(qc)

    nc.compile()
    return nc


def _prep(inputs):
    """Host-side prep: per-core in_maps (DP over batch) + build flags."""
    f8 = ml_dtypes.float8_e4m3
    x = np.asarray(inputs["x"], np.float32)
    volat = np.asarray(inputs["volatility"], np.float32)

    def to_f8(w):
        return np.ascontiguousarray(np.clip(w * WSC, -240.0, 240.0).astype(f8))

    def pack_by_out(name, n_out):
        # [K, M] -> [n_out, P, K] chunk-contiguous (per out-chunk weight block)
        w = np.asarray(inputs[name], np.float32)
        k, m = w.shape
        w4 = w.reshape(k // P, P, n_out, m // n_out)
        return to_f8(w4.transpose(2, 1, 0, 3).reshape(n_out, P, (k // P) * (m // n_out)))

    w2f = np.asarray(inputs["ffn_w2"], np.float32)
    shared = {
        "wq": pack_by_out("Wq", HC), "wk": pack_by_out("Wk", HC),
        "wv": pack_by_out("Wv", 2), "wo": pack_by_out("Wo", HC),
        "w1": pack_by_out("ffn_w1", FC),
        # w2: [HF, H] -> [2(oh), P, FC, 512]
        "w2": to_f8(w2f.reshape(FC, P, 2, 512).transpose(2, 1, 0, 3)),
        "g1": pack_by_out("gate_w1", GC), "g2": pack_by_out("gate_w2", HC),
    }
    for name, key in (("bq", "bq"), ("bk", "bk"), ("bv", "bv"), ("bo", "bo"),
                      ("b1", "ffn_b1"), ("b2", "ffn_b2"),
                      ("gb1", "gate_b1"), ("gb2", "gate_b2"),
                      ("ln1w", "ln1_w"), ("ln1b", "ln1_b"),
                      ("ln2w", "ln2_w"), ("ln2b", "ln2_b")):
        shared[name] = np.ascontiguousarray(np.asarray(inputs[key], np.float32))
    for name, key in (("gamma1", "gamma1"), ("beta1", "beta1"),
                      ("vs1w", "vs1_w"), ("vs1b", "vs1_b"),
                      ("gamma2", "gamma2"), ("beta2", "beta2"),
                      ("vs2w", "vs2_w"), ("vs2b", "vs2_b")):
        shared[name] = np.asarray(inputs[key], np.float32).reshape(1)

    flags = (
        ("bv", bool(np.any(shared["bv"]))),
        ("bo", bool(np.any(shared["bo"]))),
        ("b2", bool(np.any(shared["b2"]))),
        ("ln1w", bool(np.any(shared["ln1w"] != 1.0))),
        ("ln1b", bool(np.any(shared["ln1b"]))),
        ("beta1", bool(shared["beta1"][0] != 0.0)),
        ("ln2w", bool(np.any(shared["ln2w"] != 1.0))),
        ("ln2b", bool(np.any(shared["ln2b"]))),
        ("beta2", bool(shared["beta2"][0] != 0.0)),
    )

    in_maps = []
    for b in range(B):
        m = dict(shared)
        # x^T in [p][chunk][token] per-partition-contiguous layout
        xt = x[b].T.reshape(HC, P, S).transpose(1, 0, 2).reshape(P, HC * S)
        m["xt8"] = np.ascontiguousarray(np.clip(xt, -240.0, 240.0).astype(f8))
        m["xtf"] = np.ascontiguousarray(xt)
        m["vol"] = np.ascontiguousarray(volat[b])
        in_maps.append(m)
    return in_maps, flags


def _run(inputs, trace=False):
    in_maps, flags = _prep(inputs)
    if flags not in _BUILD_CACHE:
        _BUILD_CACHE[flags] = _build(flags)
    nc = _BUILD_CACHE[flags]
    res = run_bass_kernel_spmd(nc, in_maps, core_ids=list(range(B)), trace=trace)
    outs = np.stack([res.results[b]["out"] for b in range(B)], axis=0)
    return outs.astype(np.float32), res


def kernel(**inputs) -> np.ndarray:
    out, _ = _run(inputs, trace=False)
    return out


# revision 29
# speedup vs baseline: 1.0801x; 1.0801x over previous
"""EnhancedTransformerBlock on 8 TRN2 NeuronCores.

Strategy: pure data-parallel over batch (B=8 -> 1 element/core, no
collectives). Per core the block runs in "T-layout" ([feature, token],
features on partitions) so every matmul contracts over the partition dim.

- x is transposed on the HOST and shipped both as fp8 (matmul path) and
  f32 (residual path; DMA'd during attention).
- All weights are host-packed fp8e4 (scaled by 128) in chunk-contiguous
  DMA layouts (1-4KB per-partition lines instead of 128B descriptors).
- All weight-stationary matmuls except the attention score matmuls run as
  fp8 DoubleRow (2 k-chunks per matmul): QKV, AV, Wo, gate1/2, ffn w1/w2.
  Unscaling is folded into eviction scale= operands.
- Attention is software-pipelined: pair hp's scores/exp interleave with
  pair hp-1's AV at matmul granularity; the e-pool holds 2 pairs in
  flight so ScalarE (exp, the region bottleneck) never stalls on AV.
  Rowsum reciprocal rows are broadcast across partitions by GpSimd
  (partition_broadcast) instead of K=1 PE matmuls. ctx is carried at 16x
  in fp8 (subnormal avoidance), unscaled in the Wo eviction.
- LayerNorm1 in T-layout: column sums via ones-column matmuls, per-token
  scale/shift broadcast across partitions with K=1 matmuls.
- The residual trunk x2 (accf) is bf16; LayerNorm2 runs in N-layout after
  bf16 PE transposes (evictions alternate VectorE/ScalarE), interleaved
  per q-half with the second ffn_w2 pass so the tail overlaps matmuls.
  Typical HW exec: ~505-525us (baseline 704us); rel err ~1.03e-2.
"""
import sys

sys.path.insert(0, '/opt/trn_rl_repo')

import numpy as np
import ml_dtypes

import concourse.bass as bass
import concourse.bacc as bacc
import concourse.tile as tile
from concourse import mybir
from concourse.bass_utils import run_bass_kernel_spmd
from concourse.masks import make_identity

F32 = mybir.dt.float32
BF16 = mybir.dt.bfloat16
F8 = mybir.dt.float8e4
AF = mybir.ActivationFunctionType
OP = mybir.AluOpType
DR = mybir.MatmulPerfMode.DoubleRow

P = 128
B, S, H = 8, 1024, 1024
NH, HD = 16, 64
HF, HG = 4 * H, H // 2
HC = H // P          # 8 feature chunks
FC = HF // P         # 32 ffn chunks
GC = HG // P         # 4 gate chunks
QT = S // 512        # 2 q tiles of 512
EPS = 1e-5
WSC = 128.0          # host-side weight scale (fp8 subnormal avoidance)
WS = 1.0 / WSC
CTXS = 16.0          # ctx carried at 16x in fp8

_BUILD_CACHE = {}


def _bcast_ap(param, n_part, n_free):
    """AP reading a [n_free] DRAM tensor broadcast across n_part partitions."""
    ap = param[None, :]
    return bass.AP(tensor=ap.tensor, offset=ap.offset, ap=[[0, n_part], [1, n_free]])


def _build(flags):
    f = dict(flags)
    nc = bacc.Bacc(None, target_bir_lowering=False)

    dp = nc.declare_dram_parameter
    xt8 = dp("xt8", [P, HC * S], F8, isOutput=False)
    xtf = dp("xtf", [P, HC * S], F32, isOutput=False)
    vol = dp("vol", [S], F32, isOutput=False)
    wq = dp("wq", [HC, P, H], F8, isOutput=False)
    wk = dp("wk", [HC, P, H], F8, isOutput=False)
    wv = dp("wv", [2, P, HC * 512], F8, isOutput=False)
    wo = dp("wo", [HC, P, H], F8, isOutput=False)
    w1 = dp("w1", [FC, P, H], F8, isOutput=False)
    w2 = dp("w2", [2, P, FC, 512], F8, isOutput=False)
    g1 = dp("g1", [GC, P, H], F8, isOutput=False)
    g2 = dp("g2", [HC, P, HG], F8, isOutput=False)
    bq = dp("bq", [H], F32, isOutput=False)
    bk = dp("bk", [H], F32, isOutput=False)
    bv = dp("bv", [H], F32, isOutput=False)
    bo = dp("bo", [H], F32, isOutput=False)
    b1 = dp("b1", [HF], F32, isOutput=False)
    b2 = dp("b2", [H], F32, isOutput=False)
    gb1 = dp("gb1", [HG], F32, isOutput=False)
    gb2 = dp("gb2", [H], F32, isOutput=False)
    ln1w = dp("ln1w", [H], F32, isOutput=False)
    ln1b = dp("ln1b", [H], F32, isOutput=False)
    ln2w = dp("ln2w", [H], F32, isOutput=False)
    ln2b = dp("ln2b", [H], F32, isOutput=False)
    sc = {}
    for name in ("gamma1", "beta1", "vs1w", "vs1b", "gamma2", "beta2", "vs2w", "vs2b"):
        sc[name] = dp(name, [1], F32, isOutput=False)
    out = dp("out", [S, H], F32, isOutput=True)

    def chunked(param):  # [n] f32 -> [P, n//P] per-partition layout
        return param.rearrange("(c p) -> p c", p=P)

    with tile.TileContext(nc) as tc:
        from contextlib import ExitStack
        with ExitStack() as ctx:
            const = ctx.enter_context(tc.tile_pool(name="const", bufs=1))

            identb = const.tile([P, P], BF16)
            make_identity(nc, identb)
            ones_col = const.tile([P, 1], BF16)
            nc.vector.memset(ones_col, 1.0)
            ones_f8 = const.tile([P, 1], F8)
            nc.vector.memset(ones_f8, 1.0)
            ones_row = const.tile([1, P], BF16)
            nc.vector.memset(ones_row, 1.0)
            eps128 = const.tile([P, 1], F32)
            nc.vector.memset(eps128, EPS)

            # persistent slabs, tag-shared across phases
            trunk = ctx.enter_context(tc.tile_pool(name="trunk", bufs=1))
            xTf = trunk.tile([P, HC, S], F32, tag="f4a", name="xTf")  # x^T -> x1 -> y1
            QTs = trunk.tile([P, HC, S], BF16, tag="bf2e", name="QTs")
            KTs = trunk.tile([P, HC, S], BF16, tag="bf2b", name="KTs")
            Vp = trunk.tile([P, HC, NH, HD + 1], F8, tag="bf2c", name="Vp")
            xT8 = trunk.tile([P, HC, S], F8, tag="bf2d", name="xT8")

            # host-pretransposed x, fp8 matmul copy (per-chunk DMAs)
            for c in range(HC):
                nc.sync.dma_start(out=xT8[:, c, :], in_=xt8[:, c * S:(c + 1) * S])

            def load_chunked(param, n):
                t = const.tile([P, n], F32, name=f"c_{param.name}")
                nc.sync.dma_start(out=t, in_=chunked(param))
                return t

            bq_sb = load_chunked(bq, HC)
            bk_sb = load_chunked(bk, HC)
            bo_sb = load_chunked(bo, HC) if f["bo"] else None
            b1_sb = load_chunked(b1, FC)
            b2_sb = load_chunked(b2, HC) if f["b2"] else None
            gb1_sb = load_chunked(gb1, GC)
            gb2_sb = load_chunked(gb2, HC)
            if f["bv"]:
                bv_bc = const.tile([P, H], F32)
                nc.gpsimd.dma_start(out=bv_bc, in_=_bcast_ap(bv, P, H))
            if f["ln1w"]:
                ln1w_sb = load_chunked(ln1w, HC)
            if f["ln1b"]:
                ln1b_sb = load_chunked(ln1b, HC)
            if f["ln2w"]:
                ln2w_bc = const.tile([P, H], F32)
                nc.gpsimd.dma_start(out=ln2w_bc, in_=_bcast_ap(ln2w, P, H))
            if f["ln2b"]:
                ln2b_bc = const.tile([P, H], F32)
                nc.gpsimd.dma_start(out=ln2b_bc, in_=_bcast_ap(ln2b, P, H))

            sct = {}
            for name in ("gamma1", "vs1w", "vs1b"):
                t = const.tile([1, 1], F32, name=f"sc_{name}")
                nc.sync.dma_start(out=t, in_=sc[name][None, :])
                sct[name] = t
            for name in ("gamma2", "beta2", "vs2w", "vs2b", "beta1"):
                t = const.tile([P, 1], F32, name=f"sc_{name}")
                nc.gpsimd.dma_start(out=t, in_=_bcast_ap(sc[name], P, 1))
                sct[name] = t

            # volatility-derived per-token scales
            vol_row = const.tile([1, S], F32)
            nc.sync.dma_start(out=vol_row, in_=vol[None, :])
            s1row = const.tile([1, S], F32)
            nc.scalar.activation(s1row, vol_row, AF.Sigmoid,
                                 bias=sct["vs1b"][0:1, :], scale=sct["vs1w"][0:1, :])
            nc.vector.tensor_scalar(s1row, s1row, 1.0, sct["gamma1"],
                                    op0=OP.add, op1=OP.mult)
            vol_np = const.tile([P, HC], F32)
            nc.sync.dma_start(out=vol_np, in_=chunked(vol))
            s2_np = const.tile([P, HC], F32)
            nc.scalar.activation(s2_np, vol_np, AF.Sigmoid,
                                 bias=sct["vs2b"], scale=sct["vs2w"])
            nc.vector.tensor_scalar(s2_np, s2_np, 1.0, sct["gamma2"],
                                    op0=OP.add, op1=OP.mult)

            # ---------------- P2: Q/K projections (fp8 DoubleRow) ----------
            nc.vector.memset(Vp[:, :, :, HD:HD + 1], 1.0)
            p2w = ctx.enter_context(tc.tile_pool(name="p2w", bufs=3))
            p2wv = ctx.enter_context(tc.tile_pool(name="p2wv", bufs=2))
            ps_ctx = ExitStack()
            p23ps = ps_ctx.enter_context(
                tc.tile_pool(name="p23ps", bufs=2, space="PSUM"))

            def emit_qk_pair(mc):
                for w_par, dst, bias_sb in ((wq, QTs, bq_sb), (wk, KTs, bk_sb)):
                    wt = p2w.tile([P, HC, P], F8, tag="wproj", name="wt_qk")
                    nc.sync.dma_start(out=wt, in_=w_par[mc])
                    for qt in range(QT):
                        ps = p23ps.tile([P, 512], F32, tag="ps_qv", name="ps_qk")
                        for g in range(HC // 2):
                            nc.tensor.matmul(ps, wt[:, 2 * g:2 * g + 2, :],
                                             xT8[:, 2 * g:2 * g + 2,
                                                 qt * 512:(qt + 1) * 512],
                                             perf_mode=DR,
                                             start=(g == 0), stop=(g == HC // 2 - 1))
                        nc.vector.tensor_scalar(
                            dst[:, mc, qt * 512:(qt + 1) * 512], ps, WS,
                            bias_sb[:, mc:mc + 1], op0=OP.mult, op1=OP.add)

            def emit_v(dt):
                wt = p2wv.tile([P, HC, 512], F8, tag="wv")
                nc.sync.dma_start(out=wt, in_=wv[dt])
                for kc in range(HC):
                    ps = p23ps.tile([P, 512], F32, tag="ps_qv")
                    for g in range(HC // 2):
                        nc.tensor.matmul(ps,
                                         xT8[:, 2 * g:2 * g + 2, kc * P:(kc + 1) * P],
                                         wt[:, 2 * g:2 * g + 2, :],
                                         perf_mode=DR,
                                         start=(g == 0), stop=(g == HC // 2 - 1))
                    dst = Vp[:, kc, dt * 8:(dt + 1) * 8, 0:HD]
                    src = ps.rearrange("p (h d) -> p h d", d=HD)
                    if f["bv"]:
                        nc.vector.scalar_tensor_tensor(
                            dst, src, WS,
                            bv_bc[:, dt * 512:(dt + 1) * 512].rearrange(
                                "p (h d) -> p h d", d=HD),
                            op0=OP.mult, op1=OP.add)
                    else:
                        nc.vector.tensor_scalar(dst, src, WS, None, op0=OP.mult)

            for mc in range(HC):
                emit_qk_pair(mc)
            emit_v(0)
            # f32 x^T for the residual path (DMA runs during attention, ahead
            # of the Wo weight loads in queue order)
            for c in range(HC):
                nc.sync.dma_start(out=xTf[:, c, :], in_=xtf[:, c * S:(c + 1) * S])

            # ---------------- P3: attention (software-pipelined) -----------
            # ctxT gets its own slab: its writes start while xT8 is still
            # being read by late Q/K/V matmuls (slab sharing would WAR-block)
            ctxT = trunk.tile([P, HC, S], F8, tag="ctx", name="ctxT")
            with tc.tile_pool(name="p3e", bufs=4) as p3e, \
                 tc.tile_pool(name="p3r", bufs=2) as p3r:

                def emit_av_item(h, ep, ih, qt):
                    p0 = 64 * (h % 2)
                    pav = p23ps.tile([65, 512], F32, tag="ps_av", bufs=2)
                    for g in range(HC // 2):
                        nc.tensor.matmul(pav,
                                         Vp[:, 2 * g:2 * g + 2, h, :],
                                         ep[:, 2 * g:2 * g + 2, qt, ih, :],
                                         perf_mode=DR,
                                         start=(g == 0), stop=(g == HC // 2 - 1))
                    # reciprocal of the rowsum row, broadcast across
                    # partitions by GpSimd, then normalize (ctx at 16x)
                    rs = p3r.tile([1, 512], F32, tag="rsum")
                    nc.vector.tensor_scalar(rs, pav[64:65, :], 1.0 / CTXS,
                                            None, op0=OP.mult)
                    rrow = p3r.tile([1, 512], F32, tag="rrow")
                    nc.vector.reciprocal_approx_fast(out=rrow, in_=rs)
                    rec = p3r.tile([64, 512], F32, tag="rec")
                    nc.gpsimd.partition_broadcast(rec, rrow, channels=64)
                    nc.vector.tensor_tensor(
                        ctxT[p0:p0 + 64, h // 2, qt * 512:(qt + 1) * 512],
                        rec, pav[0:64, :], OP.mult)

                prev = None
                for hp in range(NH // 2):
                    # one e-slab per pair, laid out [kc, qt, head-in-pair, 512].
                    # Both heads of a (kc, qt) share one [P,2,512] psum tile:
                    # only the first matmul carries the WAR sem, so the second
                    # (disjoint row-group) can overlap it in the PE array.
                    ep = p3e.tile([P, HC, QT, 2, 512], F8, tag="E", name="ep",
                                  bufs=2)
                    avq = ([(prev[0] + ih, prev[1], ih, qt)
                            for ih in range(2) for qt in range(QT)]
                           if prev else [])
                    for kc in range(HC):
                        for qt in range(QT):
                            pss = p23ps.tile([P, 2, 512], F32, tag="ps_s",
                                             bufs=2, name="ps_s")
                            for i in range(2):
                                p0 = 64 * i
                                nc.tensor.matmul(
                                    pss[:, i, :],
                                    KTs[p0:p0 + 64, hp, kc * P:(kc + 1) * P],
                                    QTs[p0:p0 + 64, hp, qt * 512:(qt + 1) * 512],
                                    start=True, stop=True)
                            nc.scalar.activation(ep[:, kc, qt, :, :], pss, AF.Exp,
                                                 scale=0.125)
                        if kc % 2 == 1 and avq:
                            emit_av_item(*avq.pop(0))
                    if hp == 0:
                        emit_v(1)  # PE filler while exp(pair 0) drains
                    for item in avq:
                        emit_av_item(*item)
                    prev = (2 * hp, ep)
                for ih in range(2):
                    for qt in range(QT):
                        emit_av_item(prev[0] + ih, prev[1], ih, qt)
            ps_ctx.close()  # release P2/P3 PSUM banks before P4

            # ---------------- P4+P5: Wo + residual + LN1 + gate ----------------
            x1f8 = trunk.tile([P, HC, S], F8, tag="bf2a", name="x1f8")
            gT = trunk.tile([P, HC, S], BF16, tag="bf2e", name="gT")
            rT = trunk.tile([P, GC, S], F8, tag="bf2c", name="rT")
            g1_bufs = 1 if f["ln1b"] else 2
            with tc.tile_pool(name="pw", bufs=3) as pw:
              with tc.tile_pool(name="pt4", bufs=1) as pt4, \
                   tc.tile_pool(name="pAps", bufs=1, space="PSUM") as pAps:
                for qt in range(QT):
                    sl = slice(qt * 512, (qt + 1) * 512)
                    for mc in range(HC):
                        wt = pw.tile([P, HC, P], F8, tag="wproj", bufs=3)
                        nc.sync.dma_start(out=wt, in_=wo[mc])
                        ps = pAps.tile([P, 512], F32, tag="ps_o", bufs=2)
                        for g in range(HC // 2):
                            nc.tensor.matmul(ps, wt[:, 2 * g:2 * g + 2, :],
                                             ctxT[:, 2 * g:2 * g + 2,
                                                  qt * 512:(qt + 1) * 512],
                                             perf_mode=DR,
                                             start=(g == 0), stop=(g == HC // 2 - 1))
                        xs = xTf[:, mc, sl]
                        nc.vector.scalar_tensor_tensor(xs, ps, WS / CTXS, xs,
                                                       op0=OP.mult, op1=OP.add)
                        if f["bo"]:
                            nc.vector.tensor_scalar(xs, xs, bo_sb[:, mc:mc + 1], None,
                                                    op0=OP.add)
                        nc.scalar.activation(x1f8[:, mc, sl], xs, AF.Identity)
                    # LN1 for this q-tile; xTf: x1 -> y1 in place
                    pstat = pAps.tile([33, 512], F32, tag="ps_stat")
                    for mc in range(HC):
                        nc.tensor.matmul(pstat[0:1, :], ones_f8, x1f8[:, mc, sl],
                                         start=(mc == 0), stop=(mc == HC - 1))
                    sq = pt4.tile([P, HC, 512], BF16, tag="sq")
                    nc.scalar.activation(sq, xTf[:, :, sl], AF.Square)
                    for mc in range(HC):
                        nc.tensor.matmul(pstat[32:33, :], ones_col, sq[:, mc, :],
                                         start=(mc == 0), stop=(mc == HC - 1))
                    mu = pt4.tile([1, 512], F32, tag="mu")
                    nc.vector.tensor_scalar(mu, pstat[0:1, :], 1.0 / H, None, op0=OP.mult)
                    mu2 = pt4.tile([1, 512], F32, tag="mu2")
                    nc.vector.tensor_tensor(mu2, mu, mu, OP.mult)
                    var = pt4.tile([1, 512], F32, tag="var")
                    # var = sumsq/H - mu^2 in one op
                    nc.vector.scalar_tensor_tensor(var, pstat[32:33, :], 1.0 / H, mu2,
                                                   op0=OP.mult, op1=OP.subtract)
                    nc.scalar.activation(var, var, AF.Sqrt, bias=eps128[0:1, :])
                    rstd = pt4.tile([1, 512], F32, tag="rstd")
                    nc.vector.reciprocal_approx_fast(out=rstd, in_=var)
                    arow = pt4.tile([1, 512], F32, tag="arow")
                    nc.vector.tensor_tensor(arow, rstd, s1row[0:1, sl], OP.mult)
                    arow_bf = pt4.tile([1, 512], BF16, tag="arow_bf")
                    nc.vector.tensor_copy(arow_bf, arow)
                    crow_bf = pt4.tile([1, 512], BF16, tag="crow_bf")
                    nc.vector.tensor_tensor(crow_bf, mu, arow, OP.mult)
                    psa = pAps.tile([P, 512], F32, tag="ps_a")
                    nc.tensor.matmul(psa, ones_row, arow_bf, start=True, stop=True)
                    psc = pAps.tile([P, 512], F32, tag="ps_c")
                    nc.tensor.matmul(psc, ones_row, crow_bf, start=True, stop=True)
                    if f["ln1b"]:
                        s1_bf = pt4.tile([1, 512], BF16, tag="s1_bf")
                        nc.vector.tensor_copy(s1_bf, s1row[0:1, sl])
                        pss1 = pAps.tile([P, 512], F32, tag="ps_s1")
                        nc.tensor.matmul(pss1, ones_row, s1_bf, start=True, stop=True)
                    for mc in range(HC):
                        y = xTf[:, mc, sl]
                        nc.vector.tensor_tensor(y, y, psa, OP.mult)
                        nc.vector.tensor_tensor(y, y, psc, OP.subtract)
                        if f["ln1w"]:
                            nc.vector.tensor_scalar(y, y, ln1w_sb[:, mc:mc + 1], None,
                                                    op0=OP.mult)
                        if f["ln1b"]:
                            bs = pt4.tile([P, 512], F32, tag="bs")
                            nc.vector.tensor_scalar(bs, pss1, ln1b_sb[:, mc:mc + 1],
                                                    None, op0=OP.mult)
                            nc.vector.tensor_tensor(y, y, bs, OP.add)
                        if f["beta1"]:
                            nc.vector.tensor_scalar(y, y, sct["beta1"], None, op0=OP.add)
                        nc.scalar.activation(x1f8[:, mc, sl], y, AF.Identity)

                # gate first layer (runs while LN1 of the second q-tile drains)
                for qt in range(QT):
                    sl = slice(qt * 512, (qt + 1) * 512)
                    for mc in range(GC):
                        wt = pw.tile([P, HC, P], F8, tag="wproj", bufs=3)
                        nc.sync.dma_start(out=wt, in_=g1[mc])
                        ps = pAps.tile([P, 512], F32, tag="ps_g1", bufs=g1_bufs)
                        for g in range(HC // 2):
                            nc.tensor.matmul(ps, wt[:, 2 * g:2 * g + 2, :],
                                             x1f8[:, 2 * g:2 * g + 2, sl],
                                             perf_mode=DR,
                                             start=(g == 0), stop=(g == HC // 2 - 1))
                        nc.scalar.activation(rT[:, mc, sl], ps, AF.Relu,
                                             bias=gb1_sb[:, mc:mc + 1], scale=WS)

              y1f8 = x1f8  # fp8 y1; xTf holds f32 y1

              # ---------------- P6: gate2 + FFN + gated mix; P7 LN2 --------
              accf = trunk.tile([P, HC, S], BF16, tag="f4c", name="accf")
              with tc.tile_pool(name="pt7", bufs=2) as pt7, \
                   tc.tile_pool(name="pCps", bufs=1, space="PSUM") as pCps:
                    psk = [0]

                    def accps(shape, dtype=F32):
                        t = pCps.tile(shape, dtype, tag=f"ps_acc{psk[0] % 4}",
                                      name=f"psacc{psk[0] % 4}")
                        psk[0] += 1
                        return t

                    uT = pt7.tile([P, HC, S], BF16, tag="u", bufs=1, name="uT")
                    for qt in range(QT):
                        for mc in range(HC):
                            wt = pw.tile([P, GC, P], F8, tag="wg2", bufs=3)
                            nc.sync.dma_start(out=wt, in_=g2[mc])
                            ps = accps([P, 512])
                            for g in range(GC // 2):
                                nc.tensor.matmul(ps, wt[:, 2 * g:2 * g + 2, :],
                                                 rT[:, 2 * g:2 * g + 2,
                                                    qt * 512:(qt + 1) * 512],
                                                 perf_mode=DR,
                                                 start=(g == 0), stop=(g == GC // 2 - 1))
                            qsl = slice(qt * 512, (qt + 1) * 512)
                            nc.scalar.activation(gT[:, mc, qsl], ps,
                                                 AF.Sigmoid, bias=gb2_sb[:, mc:mc + 1],
                                                 scale=WS)
                            # u = y1*(2-g): precompute the residual half of the
                            # gated mix here, where the DVE is otherwise idle
                            tg = pt7.tile([P, 512], BF16, tag="tg", bufs=2)
                            nc.vector.tensor_scalar(tg, gT[:, mc, qsl], -1.0, 2.0,
                                                    op0=OP.mult, op1=OP.add)
                            nc.vector.tensor_tensor(uT[:, mc, qsl],
                                                    xTf[:, mc, qsl], tg, OP.mult)

                    def emit_w1(half):
                        hA = trunk.tile([P, 8, S], F8, tag="bf2b", name="hA")
                        hB = trunk.tile([P, 8, S], F8, tag="bf2d", name="hB")
                        for c in range(16):
                            cg = half * 16 + c
                            wt = pw.tile([P, HC, P], F8, tag="wproj", bufs=3)
                            nc.sync.dma_start(out=wt, in_=w1[cg])
                            psh = accps([P, S])
                            for qt in range(QT):
                                for g in range(HC // 2):
                                    nc.tensor.matmul(
                                        psh[:, qt * 512:(qt + 1) * 512],
                                        wt[:, 2 * g:2 * g + 2, :],
                                        y1f8[:, 2 * g:2 * g + 2,
                                             qt * 512:(qt + 1) * 512],
                                        perf_mode=DR,
                                        start=(g == 0), stop=(g == HC // 2 - 1))
                            dsth = hA[:, c, :] if c < 8 else hB[:, c - 8, :]
                            nc.scalar.activation(dsth, psh, AF.Gelu,
                                                 bias=b1_sb[:, cg:cg + 1], scale=WS)
                        return hA, hB

                    def hsl2(hA, hB, c, qsl):  # [P, 2, n] DoubleRow rhs slice
                        return (hA[:, c:c + 2, qsl] if c < 8
                                else hB[:, c - 8:c - 6, qsl])

                    def emit_w2_group(half, hA, hB, qt, oh):
                        qsl = slice(qt * 512, (qt + 1) * 512)
                        accs = [accps([P, 512]) for mc in range(4)]
                        for cp in range(8):
                            wt = pw.tile([P, 2, 512], F8, tag="w2", bufs=6)
                            nc.sync.dma_start(
                                out=wt,
                                in_=w2[oh, :, half * 16 + 2 * cp:half * 16 + 2 * cp + 2,
                                       :])
                            for mc in range(4):
                                nc.tensor.matmul(
                                    accs[mc],
                                    wt[:, :, mc * P:(mc + 1) * P],
                                    hsl2(hA, hB, 2 * cp, qsl),
                                    perf_mode=DR,
                                    start=(cp == 0), stop=(cp == 7))
                        for mc in range(4):
                            mcg = oh * 4 + mc
                            a = accf[:, mcg, qsl]
                            if half == 0:
                                # store ffn_half0
                                nc.vector.tensor_scalar(a, accs[mc], WS, None,
                                                        op0=OP.mult)
                            else:
                                # x2 = g*(ffn0+ffn1) + y1*(2-g); the second term
                                # (uT) was precomputed in the gate2 phase
                                af = pt7.tile([P, 512], BF16, tag="af", bufs=2)
                                nc.vector.scalar_tensor_tensor(
                                    af, accs[mc], WS, a, op0=OP.mult, op1=OP.add)
                                if f["b2"]:
                                    nc.vector.tensor_scalar(
                                        af, af, b2_sb[:, mcg:mcg + 1], None, op0=OP.add)
                                g = gT[:, mcg, qsl]
                                nc.vector.tensor_tensor(af, af, g, OP.mult)
                                nc.vector.tensor_tensor(a, af, uT[:, mcg, qsl],
                                                        OP.add)

                    def emit_ln2(qc):
                        xt = pt7.tile([P, H], BF16, tag="x2")
                        for hc in range(HC):
                            pst = accps([P, P], BF16)
                            nc.tensor.transpose(pst,
                                                accf[:, hc, qc * P:(qc + 1) * P],
                                                identb)
                            if hc % 2:
                                nc.scalar.activation(xt[:, hc * P:(hc + 1) * P],
                                                     pst, AF.Identity)
                            else:
                                nc.vector.tensor_copy(xt[:, hc * P:(hc + 1) * P],
                                                      pst)
                        stats = pt7.tile([P, 2, nc.vector.BN_STATS_DIM], F32,
                                         tag="stats")
                        for sg in range(2):
                            nc.vector.bn_stats(stats[:, sg, :],
                                               xt[:, sg * 512:(sg + 1) * 512])
                        mv = pt7.tile([P, nc.vector.BN_AGGR_DIM], F32, tag="mv")
                        nc.vector.bn_aggr(mv, stats)
                        sd = pt7.tile([P, 1], F32, tag="sd")
                        nc.scalar.activation(sd, mv[:, 1:2], AF.Sqrt, bias=eps128)
                        rstd2 = pt7.tile([P, 1], F32, tag="rstd2")
                        nc.vector.reciprocal(rstd2, sd)
                        a2 = pt7.tile([P, 1], F32, tag="a2")
                        nc.vector.tensor_tensor(a2, rstd2, s2_np[:, qc:qc + 1], OP.mult)
                        nmua = pt7.tile([P, 1], F32, tag="nmua")
                        nc.vector.scalar_tensor_tensor(nmua, mv[:, 0:1], -1.0, a2,
                                                       op0=OP.mult, op1=OP.mult)
                        ot = pt7.tile([P, H], F32, tag="ot")
                        nc.scalar.activation(ot, xt, AF.Identity, bias=nmua, scale=a2)
                        if f["ln2w"]:
                            nc.vector.tensor_tensor(ot, ot, ln2w_bc, OP.mult)
                        if f["ln2b"]:
                            bs2 = pt7.tile([P, H], F32, tag="bs2")
                            nc.vector.tensor_scalar(bs2, ln2b_bc, s2_np[:, qc:qc + 1],
                                                    None, op0=OP.mult)
                            nc.vector.tensor_tensor(ot, ot, bs2, OP.add)
                        if f["beta2"]:
                            nc.vector.tensor_scalar(ot, ot, sct["beta2"], None,
                                                    op0=OP.add)
                        nc.sync.dma_start(out=out[qc * P:(qc + 1) * P, :], in_=ot)

                    # half 0: w1 then w2 (qt-minor, unchanged)
                    hA, hB = emit_w1(0)
                    for oh in range(2):
                        for qt in range(QT):
                            emit_w2_group(0, hA, hB, qt, oh)
                    # half 1: w1, then w2 qt-major with LN2 interleaved so the
                    # first q-half's output drains while the second computes
                    hA, hB = emit_w1(1)
                    for qt in range(QT):
                        for oh in range(2):
                            emit_w2_group(1, hA, hB, qt, oh)
                        if qt == 0:
                            for qc in range(0, 4):
                                emit_ln2(qc)
                    for qc in range(4, 8):
                        emit_ln2(qc)

    nc.compile()
    return nc


def _prep(inputs):
    """Host-side prep: per-core in_maps (DP over batch) + build flags."""
    f8 = ml_dtypes.float8_e4m3
    x = np.asarray(inputs["x"], np.float32)
    volat = np.asarray(inputs["volatility"], np.float32)

    def to_f8(w):
        return np.ascontiguousarray(np.clip(w * WSC, -240.0, 240.0).astype(f8))

    def pack_by_out(name, n_out):
        # [K, M] -> [n_out, P, K] chunk-contiguous (per out-chunk weight block)
        w = np.asarray(inputs[name], np.float32)
        k, m = w.shape
        w4 = w.reshape(k // P, P, n_out, m // n_out)
        return to_f8(w4.transpose(2, 1, 0, 3).reshape(n_out, P, (k // P) * (m // n_out)))

    w2f = np.asarray(inputs["ffn_w2"], np.float32)
    shared = {
        "wq": pack_by_out("Wq", HC), "wk": pack_by_out("Wk", HC),
        "wv": pack_by_out("Wv", 2), "wo": pack_by_out("Wo", HC),
        "w1": pack_by_out("ffn_w1", FC),
        # w2: [HF, H] -> [2(oh), P, FC, 512]
        "w2": to_f8(w2f.reshape(FC, P, 2, 512).transpose(2, 1, 0, 3)),
        "g1": pack_by_out("gate_w1", GC), "g2": pack_by_out("gate_w2", HC),
    }
    for name, key in (("bq", "bq"), ("bk", "bk"), ("bv", "bv"), ("bo", "bo"),
                      ("b1", "ffn_b1"), ("b2", "ffn_b2"),
                      ("gb1", "gate_b1"), ("gb2", "gate_b2"),
                      ("ln1w", "ln1_w"), ("ln1b", "ln1_b"),
                      ("ln2w", "ln2_w"), ("ln2b", "ln2_b")):
        shared[name] = np.ascontiguousarray(np.asarray(inputs[key], np.float32))
    for name, key in (("gamma1", "gamma1"), ("beta1", "beta1"),
                      ("vs1w", "vs1_w"), ("vs1b", "vs1_b"),
                      ("gamma2", "gamma2"), ("beta2", "beta2"),
                      ("vs2w", "vs2_w"), ("vs2b", "vs2_b")):
        shared[name] = np.asarray(inputs[key], np.float32).reshape(1)

    flags = (
        ("bv", bool(np.any(shared["bv"]))),
        ("bo", bool(np.any(shared["bo"]))),
        ("b2", bool(np.any(shared["b2"]))),
        ("ln1w", bool(np.any(shared["ln1w"] != 1.0))),
        ("ln1b", bool(np.any(shared["ln1b"]))),
        ("beta1", bool(shared["beta1"][0] != 0.0)),
        ("ln2w", bool(np.any(shared["ln2w"] != 1.0))),
        ("ln2b", bool(np.any(shared["ln2b"]))),
        ("beta2", bool(shared["beta2"][0] != 0.0)),
    )

    in_maps = []
    for b in range(B):
        m = dict(shared)
        # x^T in [p][chunk][token] per-partition-contiguous layout
        xt = x[b].T.reshape(HC, P, S).transpose(1, 0, 2).reshape(P, HC * S)
        m["xt8"] = np.ascontiguousarray(np.clip(xt, -240.0, 240.0).astype(f8))
        m["xtf"] = np.ascontiguousarray(xt)
        m["vol"] = np.ascontiguousarray(volat[b])
        in_maps.append(m)
    return in_maps, flags


def _run(inputs, trace=False):
    in_maps, flags = _prep(inputs)
    if flags not in _BUILD_CACHE:
        _BUILD_CACHE[flags] = _build(flags)
    nc = _BUILD_CACHE[flags]
    res = run_bass_kernel_spmd(nc, in_maps, core_ids=list(range(B)), trace=trace)
    outs = np.stack([res.results[b]["out"] for b in range(B)], axis=0)
    return outs.astype(np.float32), res


def kernel(**inputs) -> np.ndarray:
    out, _ = _run(inputs, trace=False)
    return out
